# revision 12
# baseline (speedup 1.0000x reference)
"""Trainium2 Bass kernel for masked multi-head attention (B=4, S=2048, D=512, H=8, dk=64).

Sharding: 8 cores = 4 batches x 2 head-groups (4 heads each).
Per-core layout trick: scores are computed transposed (S^T[k, q]) so that
  - the kv mask is a per-partition bias folded into the exp activation
  - A^T feeds the AV matmul directly (no on-chip transposes anywhere)
  - an appended ones-column in V yields the softmax denominators for free
Host does the final divide-by-denominator + q-mask while unsharding.
"""

import numpy as np
import ml_dtypes

import concourse.bass as bass
import concourse.tile as tile
from concourse import bacc
from concourse import mybir
from concourse import bass2jax

# Problem constants (hardcoded per harness rules)
B, S, D = 4, 2048, 512
HEADS, DK = 8, 64
HG = 4          # heads per core (head-group)
P = 128         # partitions
NEG_BIAS = -1e9

BF16 = mybir.dt.bfloat16
F32 = mybir.dt.float32

_COMPILE_CACHE = {}


def build_bass(nkt=16, nqc=2048):
    """Build the per-core Bass graph.

    nkt: number of 128-wide k tiles to process (<= 16). Masked tail tiles can
         be skipped entirely because their exp() contribution is exactly 0.
    nqc: number of q columns to compute (<= 2048, multiple of 256).
    """
    nc = bacc.Bacc(None, target_bir_lowering=False, debug=False)

    qT = nc.declare_dram_parameter("qT", [D, S], BF16, isOutput=False)
    kT = nc.declare_dram_parameter("kT", [D, S], BF16, isOutput=False)
    vT = nc.declare_dram_parameter("vT", [D, S], BF16, isOutput=False)
    wq = nc.declare_dram_parameter("wq", [D, HG * DK], BF16, isOutput=False)
    wk = nc.declare_dram_parameter("wk", [D, HG * DK], BF16, isOutput=False)
    wv = nc.declare_dram_parameter("wv", [D, HG * DK], BF16, isOutput=False)
    kvb = nc.declare_dram_parameter("kvb", [P, S // P], F32, isOutput=False)
    out = nc.declare_dram_parameter("out", [HG * (DK + 1), S], F32, isOutput=True)

    DT = D // P           # 4 k-tiles over the D contraction
    MT = (HG * DK) // P   # 2 partition-tiles of Qp^T / Kp^T
    KT_ALL = S // P       # 16 k tiles max
    E = DK + 1            # 65: head output rows + denominator row

    # q chunking for scores/exp/AV: chunks of <=1024 columns
    q_chunks = []
    off = 0
    while off < nqc:
        w = min(1024, nqc - off)
        q_chunks.append((off, w))
        off += w

    with tile.TileContext(nc) as tc:
        with (
            tc.tile_pool(name="singles", bufs=1) as singles,
            tc.tile_pool(name="xt", bufs=3 * DT) as xt_pool,
            tc.tile_pool(name="prod", bufs=2 * MT) as prod_pool,
            tc.tile_pool(name="vp", bufs=1) as vp_pool,
            tc.tile_pool(name="aexp", bufs=3) as a_pool,
            tc.tile_pool(name="osb", bufs=2) as o_pool,
            tc.tile_pool(name="psA", bufs=2, space="PSUM") as psA,
            tc.tile_pool(name="psO", bufs=2, space="PSUM") as psO,
        ):
            # --- load weights + mask bias ---
            w_sb = {}
            for name, ap in (("wq", wq), ("wk", wk), ("wv", wv)):
                t = singles.tile([P, DT, HG * DK], BF16, tag=f"w_{name}")
                nc.sync.dma_start(out=t, in_=ap.rearrange("(t p) m -> p t m", p=P))
                w_sb[name] = t
            kvb_sb = singles.tile([P, KT_ALL], F32, tag="kvb")
            nc.sync.dma_start(out=kvb_sb, in_=kvb[:, :])

            # --- load qT/kT/vT (bf16, [512, 2048] each -> 4 tiles of [128, 2048]) ---
            x_sb = {}
            for name, ap in (("qT", qT), ("kT", kT), ("vT", vT)):
                tiles = []
                rr = ap.rearrange("(t p) n -> t p n", p=P)
                for i in range(DT):
                    t = xt_pool.tile([P, S], BF16, tag="xt")
                    nc.sync.dma_start(out=t, in_=rr[i])
                    tiles.append(t)
                x_sb[name] = tiles

            # --- projections ---
            # QpT/KpT: [256, 2048] as 2 partition-tiles, computed transposed:
            #   QpT = Wq^T @ q^T ; lhsT = Wq tile, rhs = qT tile
            qpT = [prod_pool.tile([P, S], BF16, tag="qpT", name=f"qpT{i}")
                   for i in range(MT)]
            kpT = [prod_pool.tile([P, S], BF16, tag="kpT", name=f"kpT{i}")
                   for i in range(MT)]
            for w_name, x_name, dst in (("wq", "qT", qpT), ("wk", "kT", kpT)):
                for mt in range(MT):
                    for c in range(S // 512):
                        ps = psA.tile([P, 1024], F32, tag="ps")
                        for kd in range(DT):
                            nc.tensor.matmul(
                                ps[:, :512],
                                lhsT=w_sb[w_name][:, kd, mt * P:(mt + 1) * P],
                                rhs=x_sb[x_name][kd][:, c * 512:(c + 1) * 512],
                                start=(kd == 0),
                                stop=(kd == DT - 1),
                            )
                        nc.vector.tensor_copy(
                            out=dst[mt][:, c * 512:(c + 1) * 512], in_=ps[:, :512]
                        )

            # Vp: natural layout [Sk, 4*65] with a ones column per head.
            vp = vp_pool.tile([P, KT_ALL, HG * E], BF16, tag="vp")
            nc.vector.memset(
                vp.rearrange("p t (h e) -> p t h e", e=E)[:, :, :, DK], 1.0
            )
            for mt in range(KT_ALL):
                ps = psA.tile([P, 1024], F32, tag="ps")
                for kd in range(DT):
                    nc.tensor.matmul(
                        ps[:, :HG * DK],
                        lhsT=x_sb["vT"][kd][:, mt * P:(mt + 1) * P],
                        rhs=w_sb["wv"][:, kd, :],
                        start=(kd == 0),
                        stop=(kd == DT - 1),
                    )
                nc.vector.tensor_copy(
                    out=vp[:, mt].rearrange("p (h e) -> p h e", e=E)[:, :, :DK],
                    in_=ps[:, :HG * DK].rearrange("p (h d) -> p h d", d=DK),
                )

            # --- attention per head ---
            for h in range(HG):
                mt_h = h // 2
                pb = DK * (h % 2)
                for (qoff, qw) in q_chunks:
                    ps_o = psO.tile([E, 1024], F32, tag="pso")
                    for kt in range(nkt):
                        ps_s = psA.tile([P, 1024], F32, tag="ps")
                        for (soff, sw) in ((0, min(512, qw)), (512, qw - 512)):
                            if sw <= 0:
                                continue
                            nc.tensor.matmul(
                                ps_s[:, soff:soff + sw],
                                lhsT=kpT[mt_h][pb:pb + DK, kt * P:(kt + 1) * P],
                                rhs=qpT[mt_h][pb:pb + DK, qoff + soff:qoff + soff + sw],
                                start=True,
                                stop=True,
                            )
                        a_sb = a_pool.tile([P, 1024], BF16, tag="a")
                        nc.scalar.activation(
                            out=a_sb[:, :qw],
                            in_=ps_s[:, :qw],
                            func=mybir.ActivationFunctionType.Exp,
                            bias=kvb_sb[:, kt:kt + 1],
                            scale=0.125,
                        )
                        for (soff, sw) in ((0, min(512, qw)), (512, qw - 512)):
                            if sw <= 0:
                                continue
                            nc.tensor.matmul(
                                ps_o[:, soff:soff + sw],
                                lhsT=vp[:, kt, h * E:(h + 1) * E],
                                rhs=a_sb[:, soff:soff + sw],
                                start=(kt == 0),
                                stop=(kt == nkt - 1),
                            )
                    o_sb = o_pool.tile([E, 1024], F32, tag="o")
                    nc.vector.tensor_copy(out=o_sb[:, :qw], in_=ps_o[:, :qw])
                    nc.sync.dma_start(
                        out=out[h * E:(h + 1) * E, qoff:qoff + qw],
                        in_=o_sb[:, :qw],
                    )
    nc.finalize()
    return nc


class _Runner:
    """Compile the Bass graph once and expose run()/bench() over 8 cores."""

    def __init__(self, nkt, nqc, n_cores=8):
        import jax
        from jax.experimental.shard_map import shard_map
        from jax.sharding import Mesh, PartitionSpec

        self.jax = jax
        self.n_cores = n_cores
        nc = build_bass(nkt=nkt, nqc=nqc)
        bass2jax.install_neuronx_cc_hook()
        assert nc.dbg_addr is None
        partition_name = (
            nc.partition_id_tensor.name if nc.partition_id_tensor else None
        )

        in_names, out_names, out_avals, zero_outs = [], [], [], []
        for alloc in nc.m.functions[0].allocations:
            if not isinstance(alloc, mybir.MemoryLocationSet):
                continue
            name = alloc.memorylocations[0].name
            if alloc.kind == "ExternalInput":
                if name != partition_name:
                    in_names.append(name)
            elif alloc.kind == "ExternalOutput":
                shape = tuple(alloc.tensor_shape)
                dtype = mybir.dt.np(alloc.dtype)
                out_names.append(name)
                out_avals.append(jax.core.ShapedArray(shape, dtype))
                zero_outs.append(np.zeros(shape, dtype))
        self.in_names = list(in_names)
        self.out_names = out_names
        self.zero_outs = zero_outs
        n_params = len(in_names)
        all_names = in_names + out_names
        if partition_name is not None:
            all_names = all_names + [partition_name]

        def _body(*args):
            operands = list(args)
            if partition_name is not None:
                operands.append(bass2jax.partition_id_tensor())
            outs = bass2jax._bass_exec_p.bind(
                *operands,
                out_avals=tuple(out_avals),
                in_names=tuple(all_names),
                out_names=tuple(out_names),
                lowering_input_output_aliases=(),
                sim_require_finite=True,
                sim_require_nnan=True,
                nc=nc,
            )
            return tuple(outs)

        devices = jax.devices()[:n_cores]
        self.mesh = Mesh(np.asarray(devices), ("core",))
        n_outs = len(out_names)
        in_specs = (PartitionSpec("core"),) * (n_params + n_outs)
        out_specs = (PartitionSpec("core"),) * n_outs
        donate = tuple(range(n_params, n_params + n_outs))
        mapped = shard_map(
            _body, mesh=self.mesh, in_specs=in_specs, out_specs=out_specs,
            check_rep=False,
        )
        self._run_jit = jax.jit(mapped, donate_argnums=donate, keep_unused=True)
        self._bench_jit = jax.jit(mapped, keep_unused=True)

    def _concat_inputs(self, in_maps):
        per_core = [[np.asarray(m[n]) for n in self.in_names] for m in in_maps]
        concat = [
            np.concatenate([per_core[c][i] for c in range(self.n_cores)], axis=0)
            for i in range(len(self.in_names))
        ]
        concat += [
            np.concatenate([z] * self.n_cores, axis=0) for z in self.zero_outs
        ]
        return concat

    def run(self, in_maps):
        concat = self._concat_inputs(in_maps)
        outs = self._run_jit(*concat)
        results = [{} for _ in range(self.n_cores)]
        for name, arr in zip(self.out_names, outs):
            arr = np.asarray(arr)
            per = np.split(arr, self.n_cores, axis=0)
            for c in range(self.n_cores):
                results[c][name] = per[c]
        return results

    def bench(self, in_maps, iters=30):
        import time
        jax = self.jax
        concat = [jax.device_put(x) for x in self._concat_inputs(in_maps)]
        # warm up (compiles + first exec)
        jax.block_until_ready(self._bench_jit(*concat))
        jax.block_until_ready(self._bench_jit(*concat))
        t0 = time.perf_counter()
        outs = None
        for _ in range(iters):
            outs = self._bench_jit(*concat)
        jax.block_until_ready(outs)
        t1 = time.perf_counter()
        return (t1 - t0) / iters * 1e9


def _get_compiled(nkt, nqc):
    key = (nkt, nqc)
    if key not in _COMPILE_CACHE:
        _COMPILE_CACHE[key] = _Runner(nkt, nqc)
    return _COMPILE_CACHE[key]


def _prep_in_maps(q, k, v, Wq, Wk, Wv, V_len, Q_len, nkt):
    bf = ml_dtypes.bfloat16
    in_maps = []
    for core in range(8):
        b, g = core // 2, core % 2
        cols = slice(g * HG * DK, (g + 1) * HG * DK)
        karr = np.arange(S, dtype=np.int64)
        kvb = np.where(karr < int(V_len[b]), 0.0, NEG_BIAS).astype(np.float32)
        kvb = kvb.reshape(S // P, P).T.copy()  # [128, 16]: (kt, p) -> k = kt*128+p
        in_maps.append({
            "qT": np.ascontiguousarray(q[b].T).astype(bf),
            "kT": np.ascontiguousarray(k[b].T).astype(bf),
            "vT": np.ascontiguousarray(v[b].T).astype(bf),
            "wq": np.ascontiguousarray(Wq[:, cols]).astype(bf),
            "wk": np.ascontiguousarray(Wk[:, cols]).astype(bf),
            "wv": np.ascontiguousarray(Wv[:, cols]).astype(bf),
            "kvb": np.ascontiguousarray(kvb),
        })
    return in_maps


def _postprocess(results, Q_len, nqc):
    O = np.zeros((B, S, HEADS * DK), dtype=np.float32)
    E = DK + 1
    for core in range(8):
        b, g = core // 2, core % 2
        r = np.asarray(results[core]["out"], dtype=np.float32).reshape(HG, E, S)
        num = r[:, :DK, :nqc]             # [4, 64, nqc]
        den = r[:, DK:DK + 1, :nqc]       # [4, 1, nqc]
        o = (num / den).transpose(2, 0, 1).reshape(nqc, HG * DK)
        qm = (np.arange(nqc) < int(Q_len[b])).astype(np.float32)[:, None]
        O[b, :nqc, g * HG * DK:(g + 1) * HG * DK] = o * qm
    return O


def _bounds(V_len, Q_len):
    nkt = max(1, int(min(S // P, (int(V_len.max()) + P - 1) // P)))
    nqc = max(256, int(min(S, -(-int(Q_len.max()) // 256) * 256)))
    return nkt, nqc


def _run(q, k, v, Wq, Wk, Wv, V_len, Q_len, bench=False):
    V_len = np.asarray(V_len).astype(np.int64)
    Q_len = np.asarray(Q_len).astype(np.int64)
    # JIT-specialize loop bounds to the actual masks (shared across cores).
    nkt, nqc = _bounds(V_len, Q_len)
    runner = _get_compiled(nkt, nqc)
    in_maps = _prep_in_maps(q, k, v, Wq, Wk, Wv, V_len, Q_len, nkt)
    results = runner.run(in_maps)
    out = _postprocess(results, Q_len, nqc)
    exec_ns = runner.bench(in_maps) if bench else None
    return out, exec_ns


def kernel(q, k, v, Wq, Wk, Wv, V_len, Q_len):
    q = np.asarray(q, dtype=np.float32)
    k = np.asarray(k, dtype=np.float32)
    v = np.asarray(v, dtype=np.float32)
    Wq = np.asarray(Wq, dtype=np.float32)
    Wk = np.asarray(Wk, dtype=np.float32)
    Wv = np.asarray(Wv, dtype=np.float32)
    out, _ = _run(q, k, v, Wq, Wk, Wv, V_len, Q_len, bench=False)
    return out
